# revision 1
# baseline (speedup 1.0000x reference)
"""Trainium2 Bass kernel for nn_CustomLoss_30743375905383.

loss = sum_i[ (p0-(1-t))^2 + (p1-t)^2 + 2*[wrong] ] / N
  where wrong = (t==0 ? p0<p1 : p1<p0)

Data-parallel over 8 NeuronCores: core c handles N/8 consecutive rows.
Per core, with x the interleaved pred block [p0 p1 p0 p1 ...],
d = p1-p0 and u = t*d, the partial sum decomposes into four streaming
reductions (free-dim accumulate on ScalarE/VectorE, no matmuls):

  A = sum x^2         ScalarE Square + accumulate
  B = sum p0          ScalarE Copy on even lanes + accumulate
  U = sum t*d         VectorE scalar_tensor_tensor bypass/mult + accumulate
  G = count(2u < d)   VectorE scalar_tensor_tensor mult/is_lt + accumulate

  partial = A + R - 2B - 2U + 2G      (R = rows per core)
  loss = sum(partials) / N

Each core streams its 24 MiB shard once from HBM (memory-bound); the
host combines the 8 tiny accumulator tensors in float64.
"""

import sys

if "/opt/trn_rl_repo" not in sys.path:
    sys.path.insert(0, "/opt/trn_rl_repo")

import numpy as np
import concourse.bass as bass
import concourse.mybir as mybir
import concourse.tile as tile
from concourse.bass_utils import run_bass_kernel_spmd

F32 = mybir.dt.float32
I32 = mybir.dt.int32
AF = mybir.ActivationFunctionType
ALU = mybir.AluOpType

P = 128                          # SBUF partitions
N_TOTAL = 16777216
N_CORES = 8
R = N_TOTAL // N_CORES           # rows (pairs) per core = 2097152
W = 2 * R // P                   # f32 pred elems per partition = 32768
W2 = R // P                      # int32 targets per partition = 16384

TILE_C = 4096                    # pred elems per partition per tile
IO_BUFS = 4
MID_BUFS = 2


def _split_excess_waits(nc, max_waits=1):
    """This walrus build's CoreV3 codegen caps sem-wait commands per
    instruction; split excess waits onto preceding same-engine no-ops.
    Engines run their stream in order and the waits are monotonic
    sem-ge conditions, so sequential chunked waits are equivalent."""
    counter = [0]

    def fresh_name(base):
        counter[0] += 1
        return f"{base}-wsplit{counter[0]}"

    for fn in nc.m.functions:
        for bb in fn.blocks:
            out = []
            changed = False
            for inst in bb.instructions:
                si = inst.sync_info
                waits = list(si.on_wait) if si is not None else []
                if len(waits) > max_waits:
                    changed = True
                    head, tail = waits[:-max_waits], waits[-max_waits:]
                    for i in range(0, len(head), max_waits):
                        out.append(mybir.InstNoOp(
                            name=fresh_name(inst.name),
                            sync_info=mybir.SyncInfo(
                                on_wait=head[i:i + max_waits], on_update=[]),
                            bass_nofuse=True,
                            engine=inst.engine,
                        ))
                    inst.sync_info = mybir.SyncInfo(
                        on_wait=tail, on_update=list(si.on_update))
                out.append(inst)
            if changed:
                bb.instructions = out


def _build(C=TILE_C, io_bufs=IO_BUFS, mid_bufs=MID_BUFS):
    NT = W // C
    F = C // 2
    nc = bass.Bass(trn_type="TRN2", target_bir_lowering=False, debug=False)
    pred = nc.dram_tensor("pred", [P, W], F32, kind="ExternalInput").ap()
    targ = nc.dram_tensor("targ", [P, W2], I32, kind="ExternalInput").ap()
    out_acc = nc.dram_tensor("out_acc", [P, 4 * NT], F32,
                             kind="ExternalOutput").ap()

    with tile.TileContext(nc) as tc:
        with tc.tile_pool(name="io", bufs=io_bufs) as io_pool, \
             tc.tile_pool(name="mid", bufs=mid_bufs) as mid_pool, \
             tc.tile_pool(name="accs", bufs=1) as acc_pool:
            accA = acc_pool.tile([P, NT], F32)
            accB = acc_pool.tile([P, NT], F32)
            accU = acc_pool.tile([P, NT], F32)
            accG = acc_pool.tile([P, NT], F32)
            for i in range(NT):
                X = io_pool.tile([P, C], F32, tag="X")
                T = io_pool.tile([P, F], I32, tag="T")
                # equal ~1MB chunks (X halved, T whole) keep the HWDGE
                # queues balanced and concurrently busy
                h = C // 2
                nc.sync.dma_start(X[:, :h], pred[:, i * C:i * C + h])
                nc.sync.dma_start(X[:, h:], pred[:, i * C + h:(i + 1) * C])
                nc.sync.dma_start(T[:], targ[:, i * F:(i + 1) * F])

                x2 = mid_pool.tile([P, C], F32, tag="x2")
                p0c = mid_pool.tile([P, F], F32, tag="p0c")
                dT = mid_pool.tile([P, F], F32, tag="dT")
                uT = mid_pool.tile([P, F], F32, tag="uT")
                gc = mid_pool.tile([P, F], F32, tag="gc")

                # A: sum of squares of all pred elems
                nc.scalar.activation(x2[:], X[:], AF.Square,
                                     accum_out=accA[:, i:i + 1])
                # B: sum of p0 (even lanes)
                nc.scalar.activation(p0c[:], X[:, ::2], AF.Copy,
                                     accum_out=accB[:, i:i + 1])
                # d = p1 - p0
                nc.vector.tensor_tensor(dT[:], X[:, 1::2], X[:, ::2],
                                        ALU.subtract)
                # u = t*d (int32 t converted on the fly) ; U = sum u
                nc.vector.scalar_tensor_tensor(
                    uT[:], T[:], 0.0, dT[:], ALU.bypass, ALU.mult,
                    accum_out=accU[:, i:i + 1])
                # wrong = (2u < d) ; G = count
                nc.vector.scalar_tensor_tensor(
                    gc[:], uT[:], 2.0, dT[:], ALU.mult, ALU.is_lt,
                    accum_out=accG[:, i:i + 1])

            nc.sync.dma_start(out_acc[:, 0 * NT:1 * NT], accA[:])
            nc.sync.dma_start(out_acc[:, 1 * NT:2 * NT], accB[:])
            nc.sync.dma_start(out_acc[:, 2 * NT:3 * NT], accU[:])
            nc.sync.dma_start(out_acc[:, 3 * NT:4 * NT], accG[:])

    _split_excess_waits(nc, max_waits=1)
    return nc, NT


_CACHE = {}


def _get_program():
    if "prog" not in _CACHE:
        _CACHE["prog"] = _build()
    return _CACHE["prog"]


def kernel(pred, target):
    pred = np.asarray(pred)
    target = np.asarray(target)
    assert pred.shape == (N_TOTAL, 2) and pred.dtype == np.float32
    if target.dtype != np.int32:
        # jax without x64 hands us int32; accept int64 too (values are 0/1)
        target = target.astype(np.int32)

    nc, NT = _get_program()
    in_maps = []
    for c in range(N_CORES):
        ps = pred[c * R:(c + 1) * R].reshape(P, W)
        ts = target[c * R:(c + 1) * R].reshape(P, W2)
        in_maps.append({"pred": np.ascontiguousarray(ps),
                        "targ": np.ascontiguousarray(ts)})

    res = run_bass_kernel_spmd(nc, in_maps, list(range(N_CORES)))

    total = 0.0
    for r in res.results:
        acc = np.asarray(r["out_acc"]).astype(np.float64)
        A = acc[:, 0 * NT:1 * NT].sum()
        B = acc[:, 1 * NT:2 * NT].sum()
        U = acc[:, 2 * NT:3 * NT].sum()
        G = acc[:, 3 * NT:4 * NT].sum()
        total += A + R - 2.0 * B - 2.0 * U + 2.0 * G
    return np.float32(total / N_TOTAL)

